# revision 18
# baseline (speedup 1.0000x reference)
"""Trainium2 Bass kernel for nn_CNOLReLu: bicubic 2x upsample -> leaky_relu
-> antialiased bicubic 2x downsample on a (16,128,128,128) NHWC tensor.

Data-parallel over batch: 2 images per NeuronCore.  Per channel c the op is
Y = D @ f(U @ X @ U.T) @ D.T with X = x[b,:,:,c], U = 128->256 bicubic,
D = 256->128 antialiased bicubic, f = leaky_relu(0.01).  Four matmul hops
(ping-pong layouts, no transposes):
  A: pA[w,  (ci,h2)] = X_c^T @ U^T        (data-stationary, per channel)
  B: pZ[w2, (ci,h2)] = U_t  @ sP          (matrix-stationary, per w2-chunk)
  f: Lrelu on ScalarE during PSUM->SBUF evac (bf16)
  C: pS[h2m,(ci,w')] = sA_tm^T @ D_t^T    (data-stationary, banded 8-tap D)
  D: pY[h', (w',c4)] = D_m @ sS           (matrix-stationary, 4ch packed N=512)
"""
import numpy as np
import ml_dtypes
from contextlib import ExitStack

import concourse.bacc as bacc
import concourse.tile as tile
from concourse import mybir
from concourse.bass_utils import run_bass_kernel_spmd

F32 = mybir.dt.float32
F32R = mybir.dt.float32r
BF16 = mybir.dt.bfloat16
AF = mybir.ActivationFunctionType

N_CORES = 8
B_CORE = 2          # images per core
H = W = C = 128
NEG_SLOPE = 0.01


def _keys_cubic(x):
    x = np.abs(x)
    return np.where(
        x <= 1, (1.5 * x - 2.5) * x * x + 1,
        np.where(x < 2, ((-0.5 * x + 2.5) * x - 4) * x + 2, 0.0))


def _resize_matrix(n_in, n_out):
    """Row-stochastic bicubic (antialias) resize operator; matches
    jax.image.resize(method='bicubic', antialias=True)."""
    scale = n_out / n_in
    pos = (np.arange(n_out) + 0.5) / scale - 0.5
    kscale = min(scale, 1.0)
    w = _keys_cubic((np.arange(n_in)[None, :] - pos[:, None]) * kscale)
    return (w / w.sum(axis=1, keepdims=True)).astype(np.float64)


def _band(Dm, t):
    rows = np.nonzero(np.abs(Dm[:, t * 128:(t + 1) * 128]).sum(1) > 0)[0]
    return int(rows.min()), int(rows.max()) + 1


_CACHE = {}


def _build():
    if "nc" in _CACHE:
        return _CACHE["nc"], _CACHE["consts"]

    U = _resize_matrix(H, 2 * H)       # [256,128]
    Dm = _resize_matrix(2 * H, H)      # [128,256]
    uT = U.T.astype(ml_dtypes.bfloat16)                              # [128,256]
    uT_r = U.T.astype(np.float32)
    dT = np.concatenate([Dm.T[0:128, :], Dm.T[128:256, :]], axis=1)  # [128,256]
    dT_bf = dT.astype(ml_dtypes.bfloat16)
    bands = [_band(Dm, 0), _band(Dm, 1)]   # [(0,66),(62,128)]

    nc = bacc.Bacc()
    x_d = nc.declare_dram_parameter("x", [B_CORE, H, W, C], BF16, isOutput=False)
    ut_d = nc.declare_dram_parameter("ut", [128, 256], BF16, isOutput=False)
    dbf_d = nc.declare_dram_parameter("dbf", [128, 256], BF16, isOutput=False)
    y_d = nc.declare_dram_parameter("y", [B_CORE, H, W, C], BF16, isOutput=True)

    with tile.TileContext(nc) as tc, ExitStack() as ctx:
        wpool = ctx.enter_context(tc.tile_pool(name="weights", bufs=1))
        xpool = ctx.enter_context(tc.tile_pool(name="ximg", bufs=2))
        opool = ctx.enter_context(tc.tile_pool(name="oimg", bufs=2))
        spool = ctx.enter_context(tc.tile_pool(name="stage", bufs=4))
        sapool = ctx.enter_context(tc.tile_pool(name="fine", bufs=6))
        ppool = ctx.enter_context(tc.tile_pool(name="psum2", bufs=2, space="PSUM"))

        ut_s = wpool.tile([128, 256], BF16, tag="ut")
        dbf_s = wpool.tile([128, 256], BF16, tag="dbf")
        nc.sync.dma_start(ut_s[:], ut_d[:])
        nc.sync.dma_start(dbf_s[:], dbf_d[:])

        for b in range(B_CORE):
            ximg = xpool.tile([128, W * C], BF16, tag="ximg")
            nc.sync.dma_start(ximg[:], x_d[b].rearrange("h w c -> h (w c)"))
            oimg = opool.tile([128, W * C], BF16, tag="oimg")

            for g in range(C // 4):          # 4-channel groups
                sS = spool.tile([128, 1024], BF16, tag="sS")
                pY = ppool.tile([128, 512], F32, tag="pY")
                for p in range(2):           # channel pairs in group
                    c0 = g * 4 + p * 2
                    # ---- A: pA[:, ci*256:(ci+1)*256] = X_c^T @ U^T
                    pA = ppool.tile([128, 512], F32, tag="pA")
                    for ci in range(2):
                        nc.tensor.matmul(pA[:, ci * 256:(ci + 1) * 256],
                                         ximg[:, (c0 + ci)::C], ut_s[:],
                                         start=True, stop=True)
                    sP = spool.tile([128, 512], BF16, tag="sP")
                    nc.vector.tensor_copy(sP[:], pA[:])

                    # ---- B: pZ_t = U_chunk_t @ sP ; leaky relu per chunk
                    sA = sapool.tile([128, 1024], BF16, tag="sA")
                    for t in range(2):
                        pZ = ppool.tile([128, 512], F32, tag="pZ")
                        nc.tensor.matmul(pZ[:],
                                         ut_s[:, t * 128:(t + 1) * 128],
                                         sP[:], start=True, stop=True)
                        nc.scalar.activation(sA[:, t * 512:(t + 1) * 512],
                                             pZ[:], AF.Lrelu, alpha=NEG_SLOPE)

                    # ---- C: banded W-down (bf16).
                    # pS cols = ci*256 + m*128 + w'
                    pS = ppool.tile([128, 512], F32, tag="pS")
                    for ci in range(2):
                        for m in range(2):
                            for t in range(2):
                                lo, hi = bands[t]
                                nc.tensor.matmul(
                                    pS[:, ci * 256 + m * 128 + lo:
                                       ci * 256 + m * 128 + hi],
                                    sA[:, t * 512 + ci * 256 + m * 128:
                                       t * 512 + ci * 256 + (m + 1) * 128],
                                    dbf_s[:, t * 128 + lo:t * 128 + hi],
                                    start=(t == 0), stop=(t == 1),
                                    skip_group_check=True)
                    # ---- straight evac: sS cols = (2p+ci)*256 + m*128 + w'
                    nc.vector.tensor_copy(sS[:, p * 512:(p + 1) * 512], pS[:])

                # ---- D: per channel, pY[h', c*128+w'] = sum_m D_m @ sS slice
                for cpos in range(4):
                    for m in range(2):
                        nc.tensor.matmul(
                            pY[:, cpos * 128:(cpos + 1) * 128],
                            dbf_s[:, m * 128:(m + 1) * 128],
                            sS[:, cpos * 256 + m * 128:cpos * 256 + (m + 1) * 128],
                            start=(m == 0), stop=(m == 1),
                            skip_group_check=True)
                # ---- evac pY (c4,w') -> oimg cols w'*C + c, c in group
                dsto = oimg[:].rearrange(
                    "h (w c) -> h w c", c=C)[:, :, g * 4:(g + 1) * 4]
                srco = pY[:].rearrange("h (c w) -> h w c", c=4)
                if g % 8 < 3:
                    nc.scalar.copy(dsto, srco)
                else:
                    nc.vector.tensor_copy(dsto, srco)

            nc.sync.dma_start(y_d[b].rearrange("h w c -> h (w c)"), oimg[:])

    nc.compile()
    consts = {"ut": np.ascontiguousarray(uT),
              "dbf": np.ascontiguousarray(dT_bf)}
    _CACHE["nc"] = nc
    _CACHE["consts"] = consts
    return nc, consts


def kernel(x, in_size=128, out_size=128, trace=False, tmpdir=None):
    x = np.asarray(x, dtype=np.float32)
    assert x.shape == (16, H, W, C), x.shape
    nc, consts = _build()
    in_maps = []
    for core in range(N_CORES):
        m = {"x": np.ascontiguousarray(
            x[core * B_CORE:(core + 1) * B_CORE]).astype(ml_dtypes.bfloat16)}
        m.update(consts)
        in_maps.append(m)
    res = run_bass_kernel_spmd(nc, in_maps, list(range(N_CORES)), trace=trace,
                               tmpdir=tmpdir)
    out = np.concatenate([res.results[i]["y"] for i in range(N_CORES)], axis=0)
    if trace:
        kernel.last_exec_time_ns = res.exec_time_ns
        kernel.last_results = res
    return out.astype(np.float32)


# revision 19
# speedup vs baseline: 1.0381x; 1.0381x over previous
"""Trainium2 Bass kernel for nn_CNOLReLu: bicubic 2x upsample -> leaky_relu
-> antialiased bicubic 2x downsample on a (16,128,128,128) NHWC tensor.

Data-parallel over batch: 2 images per NeuronCore.  Per channel c the op is
Y = D @ f(U @ X @ U.T) @ D.T with X = x[b,:,:,c], U = 128->256 bicubic,
D = 256->128 antialiased bicubic, f = leaky_relu(0.01).  Four matmul hops
(ping-pong layouts, no transposes):
  A: pA[w,  (ci,h2)] = X_c^T @ U^T        (data-stationary, per channel)
  B: pZ[w2, (ci,h2)] = U_t  @ sP          (matrix-stationary, per w2-chunk)
  f: Lrelu on ScalarE during PSUM->SBUF evac (bf16)
  C: pS[h2m,(ci,w')] = sA_tm^T @ D_t^T    (data-stationary, banded 8-tap D)
  D: pY[h', (w',c4)] = D_m @ sS           (matrix-stationary, 4ch packed N=512)
"""
import numpy as np
import ml_dtypes
from contextlib import ExitStack

import concourse.bacc as bacc
import concourse.tile as tile
from concourse import mybir
from concourse.bass_utils import run_bass_kernel_spmd

F32 = mybir.dt.float32
F32R = mybir.dt.float32r
BF16 = mybir.dt.bfloat16
AF = mybir.ActivationFunctionType

N_CORES = 8
B_CORE = 2          # images per core
H = W = C = 128
NEG_SLOPE = 0.01


def _keys_cubic(x):
    x = np.abs(x)
    return np.where(
        x <= 1, (1.5 * x - 2.5) * x * x + 1,
        np.where(x < 2, ((-0.5 * x + 2.5) * x - 4) * x + 2, 0.0))


def _resize_matrix(n_in, n_out):
    """Row-stochastic bicubic (antialias) resize operator; matches
    jax.image.resize(method='bicubic', antialias=True)."""
    scale = n_out / n_in
    pos = (np.arange(n_out) + 0.5) / scale - 0.5
    kscale = min(scale, 1.0)
    w = _keys_cubic((np.arange(n_in)[None, :] - pos[:, None]) * kscale)
    return (w / w.sum(axis=1, keepdims=True)).astype(np.float64)


def _band(Dm, t):
    rows = np.nonzero(np.abs(Dm[:, t * 128:(t + 1) * 128]).sum(1) > 0)[0]
    return int(rows.min()), int(rows.max()) + 1


_CACHE = {}


def _build():
    if "nc" in _CACHE:
        return _CACHE["nc"], _CACHE["consts"]

    U = _resize_matrix(H, 2 * H)       # [256,128]
    Dm = _resize_matrix(2 * H, H)      # [128,256]
    uT = U.T.astype(ml_dtypes.bfloat16)                              # [128,256]
    uT_r = U.T.astype(np.float32)
    dT = np.concatenate([Dm.T[0:128, :], Dm.T[128:256, :]], axis=1)  # [128,256]
    dT_bf = dT.astype(ml_dtypes.bfloat16)
    bands = [_band(Dm, 0), _band(Dm, 1)]   # [(0,66),(62,128)]

    nc = bacc.Bacc()
    x_d = nc.declare_dram_parameter("x", [B_CORE, H, W, C], BF16, isOutput=False)
    ut_d = nc.declare_dram_parameter("ut", [128, 256], BF16, isOutput=False)
    dbf_d = nc.declare_dram_parameter("dbf", [128, 256], BF16, isOutput=False)
    y_d = nc.declare_dram_parameter("y", [B_CORE, H, W, C], BF16, isOutput=True)

    with tile.TileContext(nc) as tc, ExitStack() as ctx:
        wpool = ctx.enter_context(tc.tile_pool(name="weights", bufs=1))
        xpool = ctx.enter_context(tc.tile_pool(name="ximg", bufs=2))
        opool = ctx.enter_context(tc.tile_pool(name="oimg", bufs=2))
        spool = ctx.enter_context(tc.tile_pool(name="stage", bufs=3))
        sapool = ctx.enter_context(tc.tile_pool(name="fine", bufs=4))
        ppool = ctx.enter_context(tc.tile_pool(name="psum2", bufs=2, space="PSUM"))

        ut_s = wpool.tile([128, 256], BF16, tag="ut")
        dbf_s = wpool.tile([128, 256], BF16, tag="dbf")
        nc.sync.dma_start(ut_s[:], ut_d[:])
        nc.sync.dma_start(dbf_s[:], dbf_d[:])

        for b in range(B_CORE):
            ximg = xpool.tile([128, W * C], BF16, tag="ximg")
            nc.sync.dma_start(ximg[:], x_d[b].rearrange("h w c -> h (w c)"))
            oimg = opool.tile([128, W * C], BF16, tag="oimg")

            for g in range(C // 4):          # 4-channel groups
                sS = spool.tile([128, 1024], BF16, tag="sS")
                pY = ppool.tile([128, 512], F32, tag="pY")
                for p in range(2):           # channel pairs in group
                    c0 = g * 4 + p * 2
                    # ---- A: pA[:, ci*256:(ci+1)*256] = X_c^T @ U^T
                    pA = ppool.tile([128, 512], F32, tag="pA")
                    for ci in range(2):
                        nc.tensor.matmul(pA[:, ci * 256:(ci + 1) * 256],
                                         ximg[:, (c0 + ci)::C], ut_s[:],
                                         start=True, stop=True)
                    sP = spool.tile([128, 512], BF16, tag="sP")
                    nc.vector.tensor_copy(sP[:], pA[:])

                    # ---- B: pZ_t = U_chunk_t @ sP ; leaky relu per chunk
                    sA = sapool.tile([128, 1024], BF16, tag="sA")
                    for t in range(2):
                        pZ = ppool.tile([128, 512], F32, tag="pZ")
                        nc.tensor.matmul(pZ[:],
                                         ut_s[:, t * 128:(t + 1) * 128],
                                         sP[:], start=True, stop=True)
                        nc.scalar.activation(sA[:, t * 512:(t + 1) * 512],
                                             pZ[:], AF.Lrelu, alpha=NEG_SLOPE)

                    # ---- C: banded W-down (bf16).
                    # pS cols = ci*256 + m*128 + w'
                    pS = ppool.tile([128, 512], F32, tag="pS")
                    for ci in range(2):
                        for m in range(2):
                            for t in range(2):
                                lo, hi = bands[t]
                                nc.tensor.matmul(
                                    pS[:, ci * 256 + m * 128 + lo:
                                       ci * 256 + m * 128 + hi],
                                    sA[:, t * 512 + ci * 256 + m * 128:
                                       t * 512 + ci * 256 + (m + 1) * 128],
                                    dbf_s[:, t * 128 + lo:t * 128 + hi],
                                    start=(t == 0), stop=(t == 1),
                                    skip_group_check=True)
                    # ---- straight evac: sS cols = (2p+ci)*256 + m*128 + w'
                    nc.vector.tensor_copy(sS[:, p * 512:(p + 1) * 512], pS[:])

                # ---- D: per channel, pY[h', c*128+w'] = sum_m D_m @ sS slice
                for cpos in range(4):
                    for m in range(2):
                        nc.tensor.matmul(
                            pY[:, cpos * 128:(cpos + 1) * 128],
                            dbf_s[:, m * 128:(m + 1) * 128],
                            sS[:, cpos * 256 + m * 128:cpos * 256 + (m + 1) * 128],
                            start=(m == 0), stop=(m == 1),
                            skip_group_check=True)
                # ---- evac pY (c4,w') -> oimg cols w'*C + c, c in group
                dsto = oimg[:].rearrange(
                    "h (w c) -> h w c", c=C)[:, :, g * 4:(g + 1) * 4]
                srco = pY[:].rearrange("h (c w) -> h w c", c=4)
                if g % 2 == 0:
                    nc.scalar.copy(dsto, srco)
                else:
                    nc.vector.tensor_copy(dsto, srco)

            nc.sync.dma_start(y_d[b].rearrange("h w c -> h (w c)"), oimg[:])

    nc.compile()
    consts = {"ut": np.ascontiguousarray(uT),
              "dbf": np.ascontiguousarray(dT_bf)}
    _CACHE["nc"] = nc
    _CACHE["consts"] = consts
    return nc, consts


def kernel(x, in_size=128, out_size=128, trace=False, tmpdir=None):
    x = np.asarray(x, dtype=np.float32)
    assert x.shape == (16, H, W, C), x.shape
    nc, consts = _build()
    in_maps = []
    for core in range(N_CORES):
        m = {"x": np.ascontiguousarray(
            x[core * B_CORE:(core + 1) * B_CORE]).astype(ml_dtypes.bfloat16)}
        m.update(consts)
        in_maps.append(m)
    res = run_bass_kernel_spmd(nc, in_maps, list(range(N_CORES)), trace=trace,
                               tmpdir=tmpdir)
    out = np.concatenate([res.results[i]["y"] for i in range(N_CORES)], axis=0)
    if trace:
        kernel.last_exec_time_ns = res.exec_time_ns
        kernel.last_results = res
    return out.astype(np.float32)
